# revision 1
# baseline (speedup 1.0000x reference)
"""Trainium2 Bass kernel for nn_MultiHeadAttention_81673098101666.

Reference computation (per batch b):
    qkv  = seq @ w_qkv.T ; q,k,v = split(qkv)        # seq [S,128], q/k/v [S,1024]
    scores = q @ k.T / 32 ; attn = softmax(scores)
    out  = attn @ v @ w_out.T + b_out                # [S, 128]

Key algebraic identity (INPUT_DIM=128 => rank-128 attention):
    scores^T = seq (Wk^T Wq) seq_q^T      with M  = Wk^T Wq   [128,128]
    out^T    = W2T^T (seq^T E^T) / sumexp with W2T = Wv^T Wout^T [128,128]
so the S^2-sized matmuls contract over 128 dims instead of 1024 (8x fewer
FLOPs) and Q/K/V are never materialized.

Sharding: 8 cores = 4 batches x 2 query-halves; no collectives. Each core
returns the unnormalized projected context (outT, [128, 1024]) plus the
softmax denominator; the host divides and adds the bias.

Device layouts (P=partition dim first):
    seqT  [128(i), 2048(k)]   seqq^T [128(i), 1024(q)]   seq_nat [k, i] tiles
    A = M^T-contracted seq: A[j, k] (lhsT for scores)
    ET[k, q] = exp(scoresT/32)
    C[i, q] = seq^T E^T ;  outT[c, q] = W2T^T C

All matmul operands are float32r (fp32 rounded to 11-bit mantissa, full-rate
PE path).
"""

import numpy as np

B, S, DIN = 4, 2048, 128
O = 1024
QPC = S // 2           # queries per core = 1024
QC = 512               # query-chunk width
N_CHUNK = QPC // QC    # 2
NKT = S // 128         # 16 key tiles
ND = O // 128          # 8 o-tiles (weight contraction)
SCALE = 1.0 / 32.0     # 1/sqrt(O)

_NC = None
PROFILE = False
LAST_RESULTS = None


def _body(ctx, tc, seqkv, seqn, seqq, Md, W2Td, outT_d, sumexp_d):
    import concourse.mybir as mybir

    nc = tc.nc
    f32 = mybir.dt.float32
    f32r = mybir.dt.float32r

    consts = ctx.enter_context(tc.tile_pool(name="consts", bufs=1))
    et_pool = ctx.enter_context(tc.tile_pool(name="et", bufs=16))
    c_pool = ctx.enter_context(tc.tile_pool(name="cp", bufs=2))
    out_pool = ctx.enter_context(tc.tile_pool(name="outs", bufs=4))
    psum = ctx.enter_context(tc.tile_pool(name="psum", bufs=1, space="PSUM"))

    # ---- loads: wave 1 = M/W2T + seqkv + seqq (feeds A -> scores -> exp);
    # seqn (only needed by C, ~10us later) queues behind on the same HW queues.
    M_sb = consts.tile([128, 128], f32r)       # M = Wk^T Wq (host-computed)
    nc.sync.dma_start(M_sb[:], Md[:].bitcast(f32r))
    W2T_sb = consts.tile([128, 128], f32r)     # W2T = Wv^T Wout^T (host)
    nc.sync.dma_start(W2T_sb[:], W2Td[:].bitcast(f32r))
    seqkv_sb = consts.tile([128, S], f32r)
    seqq_sb = consts.tile([128, QPC], f32r)
    for c in range(2):
        nc.sync.dma_start(seqkv_sb[:, c * 256:(c + 1) * 256],
                          seqkv[:, c * 256:(c + 1) * 256].bitcast(f32r))
    for c in range(4):
        nc.sync.dma_start(seqq_sb[:, c * 256:(c + 1) * 256],
                          seqq[:, c * 256:(c + 1) * 256].bitcast(f32r))
    for c in range(2, 8):
        nc.sync.dma_start(seqkv_sb[:, c * 256:(c + 1) * 256],
                          seqkv[:, c * 256:(c + 1) * 256].bitcast(f32r))
    seqn_sb = consts.tile([128, NKT * 128], f32r)  # seq natural [k, i], 16 tiles
    sn3 = seqn.bitcast(f32r).rearrange("(t p) i -> p t i", p=128)  # [128, 16, 128]
    snsb3 = seqn_sb[:].rearrange("p (t i) -> p t i", i=128)
    for t0, t1 in ((0, 4), (4, 8), (8, 12), (12, 16)):
        nc.sync.dma_start(snsb3[:, t0:t1, :], sn3[:, t0:t1, :])

    ones_f = consts.tile([128, 2], f32)
    nc.any.memset(ones_f[:], 1.0)
    ones_col = consts.tile([128, 2], f32r)
    nc.scalar.copy(ones_col[:], ones_f[:])

    A_sb = consts.tile([128, S], f32r)     # A[j, k]
    for ac in range(4):
        pa = psum.tile([128, 512], f32, tag="ctx", bufs=2)
        nc.tensor.matmul(pa[:], M_sb[:], seqkv_sb[:, ac * 512:(ac + 1) * 512],
                         start=True, stop=True)
        nc.vector.tensor_copy(A_sb[:, ac * 512:(ac + 1) * 512], pa[:])

    # ---- scores + exp for BOTH query chunks at once ---------------------
    ets = []
    for kt in range(NKT):
        pp = psum.tile([128, 1024], f32, tag="mm", bufs=2)
        for qc in range(N_CHUNK):
            nc.tensor.matmul(pp[:, qc * QC:(qc + 1) * QC],
                             A_sb[:, kt * 128:(kt + 1) * 128],
                             seqq_sb[:, qc * QC:(qc + 1) * QC],
                             start=True, stop=True, skip_group_check=True)
        et = et_pool.tile([128, 1024], f32r, tag="et")
        nc.scalar.activation(et[:], pp[:],
                             mybir.ActivationFunctionType.Exp, scale=float(SCALE))
        ets.append(et)

    # ---- C + sumexp accumulation for both chunks, interleaved per kt ----
    # (each kt-step consumes ets[kt] as soon as the exp chain produces it)
    pcs = []
    pses = []
    for qc in range(N_CHUNK):
        pcs.append(psum.tile([128, QC], f32, tag="ctx", bufs=2, name=f"pc{qc}"))
        pses.append(psum.tile([2, QC], f32, tag="aux", bufs=2, name=f"pse{qc}"))
    for kt in range(NKT):
        for qc in range(N_CHUNK):
            q0 = qc * QC
            nc.tensor.matmul(pcs[qc][:], seqn_sb[:, kt * 128:(kt + 1) * 128],
                             ets[kt][:, q0: q0 + QC],
                             start=(kt == 0), stop=(kt == NKT - 1))
            nc.tensor.matmul(pses[qc][:], ones_col[:], ets[kt][:, q0: q0 + QC],
                             start=(kt == 0), stop=(kt == NKT - 1))

    for qc in range(N_CHUNK):
        q0 = qc * QC
        C_sb = c_pool.tile([128, QC], f32r, tag="c")
        nc.vector.tensor_copy(C_sb[:], pcs[qc][:])
        se_sb = out_pool.tile([1, QC], f32, tag="se_sb")
        nc.vector.tensor_copy(se_sb[:], pses[qc][:1, :])
        nc.sync.dma_start(sumexp_d[:, q0: q0 + QC], se_sb[:])
        po = psum.tile([128, QC], f32, tag="mm", bufs=2)
        nc.tensor.matmul(po[:], W2T_sb[:], C_sb[:], start=True, stop=True)
        ot = out_pool.tile([128, QC], f32, tag="ot")
        nc.vector.tensor_copy(ot[:], po[:])
        nc.sync.dma_start(outT_d[:, q0: q0 + QC], ot[:])


def _build_nc():
    from contextlib import ExitStack

    import concourse.mybir as mybir
    import concourse.tile as tile
    from concourse import bacc

    f32 = mybir.dt.float32
    nc = bacc.Bacc("TRN2", target_bir_lowering=False, debug=False, num_devices=8)
    seqkv = nc.dram_tensor("seqT_kv", [128, S], f32, kind="ExternalInput").ap()
    seqn = nc.dram_tensor("seq_nat", [S, 128], f32, kind="ExternalInput").ap()
    seqq = nc.dram_tensor("seqT_q", [128, QPC], f32, kind="ExternalInput").ap()
    Md = nc.dram_tensor("M_in", [128, 128], f32, kind="ExternalInput").ap()
    W2Td = nc.dram_tensor("W2T_in", [128, 128], f32, kind="ExternalInput").ap()
    outT_d = nc.dram_tensor("outT", [128, QPC], f32, kind="ExternalOutput").ap()
    sumexp_d = nc.dram_tensor("sumexp", [1, QPC], f32, kind="ExternalOutput").ap()

    with tile.TileContext(nc) as tc:
        with ExitStack() as ctx:
            _body(ctx, tc, seqkv, seqn, seqq, Md, W2Td, outT_d, sumexp_d)
    nc.compile()
    return nc


def get_nc():
    global _NC
    if _NC is None:
        _NC = _build_nc()
    return _NC


def make_in_maps(sequence, w_qkv, w_out):
    seqT = np.ascontiguousarray(np.transpose(sequence, (0, 2, 1)))  # [B, 128, S]
    wq, wk, wv = w_qkv[:O], w_qkv[O:2 * O], w_qkv[2 * O:]
    M = np.ascontiguousarray(wk.T @ wq)            # [128, 128]
    W2T = np.ascontiguousarray(wv.T @ w_out.T)     # [128, 128]
    in_maps = []
    for c in range(8):
        b, h = c // 2, c % 2
        in_maps.append({
            "seqT_kv": seqT[b],
            "seq_nat": np.ascontiguousarray(sequence[b]),
            "seqT_q": np.ascontiguousarray(seqT[b][:, h * QPC:(h + 1) * QPC]),
            "M_in": M,
            "W2T_in": W2T,
        })
    return in_maps


def kernel(sequence, w_qkv, w_out, b_out):
    global LAST_RESULTS
    from concourse.bass_utils import run_bass_kernel_spmd

    sequence = np.asarray(sequence, dtype=np.float32)
    w_qkv = np.asarray(w_qkv, dtype=np.float32)
    w_out = np.asarray(w_out, dtype=np.float32)
    b_out = np.asarray(b_out, dtype=np.float32)

    nc = get_nc()
    in_maps = make_in_maps(sequence, w_qkv, w_out)
    kw = {}
    if PROFILE:
        kw = dict(trace=True, trace_cores=[0])
    res = run_bass_kernel_spmd(nc, in_maps, list(range(8)), **kw)
    LAST_RESULTS = res

    out = np.empty((B, S, DIN), np.float32)
    for c in range(8):
        b, h = c // 2, c % 2
        outT = res.results[c]["outT"]          # [128, 1024] unnormalized c-major
        se = res.results[c]["sumexp"][0]       # [1024]
        out[b, h * QPC:(h + 1) * QPC, :] = outT.T / se[:, None] + b_out[None, :]
    return out



# revision 9
# speedup vs baseline: 1.2766x; 1.2766x over previous
"""Trainium2 Bass kernel for nn_MultiHeadAttention_81673098101666.

Reference computation (per batch b):
    qkv  = seq @ w_qkv.T ; q,k,v = split(qkv)        # seq [S,128], q/k/v [S,1024]
    scores = q @ k.T / 32 ; attn = softmax(scores)
    out  = attn @ v @ w_out.T + b_out                # [S, 128]

Key algebraic identity (INPUT_DIM=128 => rank-128 attention):
    scoresT = (M^T seqT)^T-contracted against seq_q   with M = Wk^T Wq [128,128]
    outT    = W2T^T (seqT E^T) / sumexp               with W2T = Wv^T Wout^T
so the S^2-sized matmuls contract over 128 dims instead of 1024 and Q/K/V
are never materialized.

Sharding: 8 cores = 4 batches x 2 query-halves; no collectives. Each core's
seqkv columns are PERMUTED so its own query half comes first; attention is
permutation-invariant over keys, so A/scores/C just see reordered keys.
This makes "seqq" a plain slice of seqkv (no separate load).

v2 changes vs v1 (47.2us -> target ~25us):
  - all matmul operands bf16 (host-cast): halves DMA bytes; FWL weight loads
  - 4 big contiguous input DMAs split across the two HW DGE queues
    (sync + scalar) instead of 22 small ones on sync only (was an 11.5us
    serial head at 90GB/s)
  - sumexp: 32 ones-matmuls (7.5us of PE) replaced by a DVE add-chain over
    the exp tiles + 4 tiny ones-matmuls accumulated in PSUM
  - exp chain is the critical path (scalar engine, ~16us of Exp): scores
    are emitted early so exp[0] starts ~3us in; everything else overlaps
  - tail split across engines: qc0 output path on scalar, qc1 on vector,
    outputs in bf16 on both DMA queues
"""

import numpy as np

B, S, DIN = 4, 2048, 128
O = 1024
QPC = S // 2           # queries per core = 1024
QC = 512               # query-chunk width (PSUM bank limit: 512 fp32)
NKT = S // 128         # 16 key tiles
SCALE = 1.0 / 32.0     # 1/sqrt(O)

_NC = None
PROFILE = False
LAST_RESULTS = None


def _body(ctx, tc, in1, in2, in3, in4, outT_d, sumexp_d):
    import concourse.mybir as mybir

    nc = tc.nc
    f32 = mybir.dt.float32
    b16 = mybir.dt.bfloat16
    Exp = mybir.ActivationFunctionType.Exp

    consts = ctx.enter_context(tc.tile_pool(name="consts", bufs=1))
    et_pool = ctx.enter_context(tc.tile_pool(name="et", bufs=16))
    acc_pool = ctx.enter_context(tc.tile_pool(name="accp", bufs=2))
    c_pool = ctx.enter_context(tc.tile_pool(name="cp", bufs=2))
    out_pool = ctx.enter_context(tc.tile_pool(name="outs", bufs=4))
    psum = ctx.enter_context(tc.tile_pool(name="psum", bufs=1, space="PSUM"))

    # ---- SBUF tiles ----------------------------------------------------
    # in1 = M || seqT query-half ; in2 = W2T || seqT key-half
    in1_sb = consts.tile([128, 128 + QPC], b16)
    in2_sb = consts.tile([128, 128 + QPC], b16)
    seqn_sb = consts.tile([128, S], b16)       # seq natural, [p, t*128+i]
    A_sb = consts.tile([128, S], b16)          # A[j,k] = (M^T seqT)[j,k]
    ones_f = consts.tile([128, 2], f32)
    ones_sb = consts.tile([128, 2], b16)
    warm_sb = consts.tile([128, QC], b16)

    # ---- input DMAs: sync + scalar HW queues, gpsimd SW queue ----------
    nc.sync.dma_start(in1_sb[:], in1[:])
    nc.sync.dma_start(seqn_sb[:, 0:QPC], in3[:])
    nc.scalar.dma_start(in2_sb[:], in2[:])
    nc.gpsimd.dma_start(seqn_sb[:, QPC:S], in4[:])

    # warm-up matmuls: keep PE busy through the DMA head so the HAM
    # clock-gate releases (1.2 -> 2.4 GHz) before the real stream starts
    nc.vector.memset(warm_sb[:], 1.0)
    for w in range(9):
        pw = psum.tile([128, QC], f32, tag="mm", bufs=2, name=f"pw{w}")
        nc.tensor.matmul(pw[:], warm_sb[:, 0:128], warm_sb[:],
                         start=True, stop=True)

    nc.vector.memset(ones_f[:], 1.0)
    nc.vector.tensor_copy(ones_sb[:], ones_f[:])

    # ---- A = M^T seqT, then scores ------------------------------------
    def a_chunk(ac):
        src = in1_sb if ac < 2 else in2_sb
        pa = psum.tile([128, QC], f32, tag="aux", bufs=2, name=f"pa{ac}")
        nc.tensor.matmul(pa[:], in1_sb[:, 0:128],
                         src[:, 128 + (ac % 2) * QC:128 + (ac % 2 + 1) * QC],
                         start=True, stop=True)
        nc.vector.tensor_copy(A_sb[:, ac * QC:(ac + 1) * QC], pa[:])

    ets = []

    def score_tile(kt):
        pp = psum.tile([128, 1024], f32, tag="mm", bufs=2, name=f"pp{kt}")
        for qc in range(2):
            nc.tensor.matmul(pp[:, qc * QC:(qc + 1) * QC],
                             A_sb[:, kt * 128:(kt + 1) * 128],
                             in1_sb[:, 128 + qc * QC:128 + (qc + 1) * QC],
                             start=True, stop=True, skip_group_check=True)
        et = et_pool.tile([128, 1024], b16, tag="et", name=f"et{kt}")
        nc.scalar.activation(et[:], pp[:], Exp, scale=float(SCALE))
        ets.append(et)

    a_chunk(0)
    a_chunk(1)
    for kt in range(8):
        score_tile(kt)
    a_chunk(2)
    a_chunk(3)
    for kt in range(8, NKT):
        score_tile(kt)

    # ---- DVE: accumulate exp tiles for sumexp (two chains) -------------
    accA = acc_pool.tile([128, 1024], b16, tag="acc", name="accA")
    accB = acc_pool.tile([128, 1024], b16, tag="acc", name="accB")
    add = mybir.AluOpType.add
    nc.vector.tensor_tensor(accA[:], ets[0][:], ets[1][:], add)
    nc.vector.tensor_tensor(accB[:], ets[2][:], ets[3][:], add)
    for kt in range(4, NKT):
        dst = accA if kt % 2 == 0 else accB
        nc.vector.tensor_tensor(dst[:], dst[:], ets[kt][:], add)

    # ---- C accumulation (both query chunks, interleaved per kt) --------
    pcs = [psum.tile([128, QC], f32, tag="ctx", bufs=2, name=f"pc{qc}")
           for qc in range(2)]
    for kt in range(NKT):
        for qc in range(2):
            nc.tensor.matmul(pcs[qc][:], seqn_sb[:, kt * 128:(kt + 1) * 128],
                             ets[kt][:, qc * QC:(qc + 1) * QC],
                             start=(kt == 0), stop=(kt == NKT - 1))

    # ---- sumexp = ones^T (accA + accB) via PSUM accumulation -----------
    pses = []
    for h in range(2):
        pse = psum.tile([2, QC], f32, tag="mm", bufs=2, name=f"pse{h}")
        nc.tensor.matmul(pse[:1, :], ones_sb[:, 0:1], accA[:, h * QC:(h + 1) * QC],
                         start=True, stop=False)
        nc.tensor.matmul(pse[:1, :], ones_sb[:, 0:1], accB[:, h * QC:(h + 1) * QC],
                         start=False, stop=True)
        pses.append(pse)

    # ---- output projection: qc0 path on scalar, qc1 path on vector -----
    # qc0
    C0_sb = c_pool.tile([128, QC], b16, tag="c", name="C0")
    nc.scalar.copy(C0_sb[:], pcs[0][:])
    po0 = psum.tile([128, QC], f32, tag="aux", bufs=2, name="po0")
    nc.tensor.matmul(po0[:], in2_sb[:, 0:128], C0_sb[:], start=True, stop=True)
    ot0 = out_pool.tile([128, QC], b16, tag="ot", name="ot0")
    nc.scalar.copy(ot0[:], po0[:])
    nc.scalar.dma_start(outT_d[:, 0:QC], ot0[:])

    # qc1
    C1_sb = c_pool.tile([128, QC], b16, tag="c", name="C1")
    nc.vector.tensor_copy(C1_sb[:], pcs[1][:])
    po1 = psum.tile([128, QC], f32, tag="aux", bufs=2, name="po1")
    nc.tensor.matmul(po1[:], in2_sb[:, 0:128], C1_sb[:], start=True, stop=True)
    ot1 = out_pool.tile([128, QC], b16, tag="ot", name="ot1")
    nc.vector.tensor_copy(ot1[:], po1[:])
    nc.sync.dma_start(outT_d[:, QC:2 * QC], ot1[:])

    # sumexp out: one copy on each of scalar/vector, DMA on sync
    se_sb = out_pool.tile([1, 2 * QC], f32, tag="se", name="se")
    nc.scalar.copy(se_sb[:, 0:QC], pses[0][:1, :])
    nc.vector.tensor_copy(se_sb[:, QC:2 * QC], pses[1][:1, :])
    nc.sync.dma_start(sumexp_d[:], se_sb[:])


def _build_nc():
    from contextlib import ExitStack

    import concourse.mybir as mybir
    import concourse.tile as tile
    from concourse import bacc

    f32 = mybir.dt.float32
    b16 = mybir.dt.bfloat16
    nc = bacc.Bacc("TRN2", target_bir_lowering=False, debug=False, num_devices=8)
    in1 = nc.dram_tensor("in1", [128, 128 + QPC], b16, kind="ExternalInput").ap()
    in2 = nc.dram_tensor("in2", [128, 128 + QPC], b16, kind="ExternalInput").ap()
    in3 = nc.dram_tensor("in3", [128, QPC], b16, kind="ExternalInput").ap()
    in4 = nc.dram_tensor("in4", [128, QPC], b16, kind="ExternalInput").ap()
    outT_d = nc.dram_tensor("outT", [128, QPC], b16, kind="ExternalOutput").ap()
    sumexp_d = nc.dram_tensor("sumexp", [1, QPC], f32, kind="ExternalOutput").ap()

    with tile.TileContext(nc) as tc:
        with ExitStack() as ctx:
            _body(ctx, tc, in1, in2, in3, in4, outT_d, sumexp_d)
    nc.compile()
    return nc


def get_nc():
    global _NC
    if _NC is None:
        _NC = _build_nc()
    return _NC


def make_in_maps(sequence, w_qkv, w_out):
    import ml_dtypes

    bf16 = ml_dtypes.bfloat16
    wq, wk, wv = w_qkv[:O], w_qkv[O:2 * O], w_qkv[2 * O:]
    M = (wk.T @ wq).astype(bf16)                   # [128, 128]
    W2T = (wv.T @ w_out.T).astype(bf16)            # [128, 128]

    in_maps = []
    for c in range(8):
        b, h = c // 2, c % 2
        seq = sequence[b]
        if h == 1:  # query half first; attention is permutation-inv over keys
            seq = np.concatenate([seq[QPC:], seq[:QPC]], axis=0)
        seq16 = seq.astype(bf16)                   # [2048, 128]
        seqT = np.ascontiguousarray(seq16.T)       # [128, 2048]
        # seqn tiled: partition p holds [t, i] for key t*128+p
        seqn = np.ascontiguousarray(
            seq16.reshape(NKT, 128, 128).transpose(1, 0, 2).reshape(128, S))
        in_maps.append({
            "in1": np.ascontiguousarray(np.concatenate([M, seqT[:, :QPC]], axis=1)),
            "in2": np.ascontiguousarray(np.concatenate([W2T, seqT[:, QPC:]], axis=1)),
            "in3": np.ascontiguousarray(seqn[:, :QPC]),
            "in4": np.ascontiguousarray(seqn[:, QPC:]),
        })
    return in_maps


def kernel(sequence, w_qkv, w_out, b_out):
    global LAST_RESULTS
    from concourse.bass_utils import run_bass_kernel_spmd

    sequence = np.asarray(sequence, dtype=np.float32)
    w_qkv = np.asarray(w_qkv, dtype=np.float32)
    w_out = np.asarray(w_out, dtype=np.float32)
    b_out = np.asarray(b_out, dtype=np.float32)

    nc = get_nc()
    in_maps = make_in_maps(sequence, w_qkv, w_out)
    kw = {}
    if PROFILE:
        kw = dict(trace=True, trace_cores=[0])
    res = run_bass_kernel_spmd(nc, in_maps, list(range(8)), **kw)
    LAST_RESULTS = res

    out = np.empty((B, S, DIN), np.float32)
    for c in range(8):
        b, h = c // 2, c % 2
        outT = res.results[c]["outT"].astype(np.float32)   # [128, 1024]
        se = res.results[c]["sumexp"].reshape(QPC)         # [1024]
        out[b, h * QPC:(h + 1) * QPC, :] = outT.T / se[:, None] + b_out[None, :]
    return out
